# revision 29
# baseline (speedup 1.0000x reference)
"""Trainium2 Bass kernel for nn_EmbeddingEncoder (dense transformer encoder).

Strategy (8 cores, data-parallel over batch, 16 batches/core):
- Canonical activation layout: channels-first [96, tokens] in SBUF, with
  6-col zero guards between batches (+3 outer) so the depthwise conv's
  shifted windows never cross batch boundaries.
- All matmuls in float32r (TF32-like, 1 cyc/row at N>=256).
- LN with zero-mean weight folding: every consumer weight matrix M of the
  LN output is column-centered on the host (M' = P M, P = I - 11^T/D), so
  mean subtraction never happens on device; LN = x * rstd_broadcast only.
  Stats via ones-column matmuls -> [13,480] compact tiles; rstd broadcast
  back via K=13 selector matmuls; reciprocal via fast-approx DVE op.
- Conv block: depthwise+pointwise fused into 7 per-tap [96,96] matrices
  M_k = pw^T * dw_k (host-precomputed, column-centered), 7 accumulating
  matmuls per chunk.
- Attention: scores computed transposed ([k,q]); true per-query max via
  gpsimd partition_all_reduce(max) on the score PSUM chunks (no LSE bound,
  no Ln, single Exp pass -> no activation-table thrash); -max applied into
  the open score PSUM group by rank-1 ones matmuls; softmax denominator Z
  piggybacks as a 97th all-ones column on the V matrix so the ctx matmul
  computes it for free; 1/Z via fast-approx reciprocal, broadcast to the
  96 ctx rows via gpsimd partition_broadcast.
"""
import sys
import math

sys.path.insert(0, "/opt/trn_rl_repo")

import numpy as np

B, S, D, H, KW, L = 128, 384, 96, 4, 7, 4
NCORES = 8
BL = B // NCORES            # 16 batches per core
TOK = BL * S                # 6144 tokens per core
STRIDE = S + 6              # 390: batch stride in padded layout
PADW = 3 + BL * STRIDE - 6 + 3  # data width 6240
TILEW = PADW + 6            # 6246 incl 3-col outer guards both sides
NCH = 13                    # LN/conv/ffn chunking
CHW = 480                   # 13*480 = 6240
SQ96 = math.sqrt(96.0)

_cache = {}


def _build_module():
    import concourse.bass as bass
    import concourse.bacc as bacc
    import concourse.mybir as mybir
    import concourse.tile as tile
    import concourse.bass_isa as bass_isa

    f32 = mybir.dt.float32
    f32r = mybir.dt.float32r
    AF = mybir.ActivationFunctionType
    ALU = mybir.AluOpType

    nc = bacc.Bacc("TRN2", target_bir_lowering=False)

    # ---- DRAM tensors ----
    xin = nc.dram_tensor("xin", [TOK, D], f32r, kind="ExternalInput")
    peT = nc.dram_tensor("peT", [D, S], f32r, kind="ExternalInput")
    eye = nc.dram_tensor("eye", [128, 128], f32r, kind="ExternalInput")
    ones = nc.dram_tensor("ones", [128, 128], f32r, kind="ExternalInput")
    ejst = nc.dram_tensor("ejst", [NCH, D, NCH], f32r, kind="ExternalInput")
    bsel = nc.dram_tensor("bsel", [NCH, NCH, D], f32r, kind="ExternalInput")
    mk = nc.dram_tensor("mk", [L, KW, D, D], f32r, kind="ExternalInput")
    cbias = nc.dram_tensor("cbias", [D, L], f32, kind="ExternalInput")
    gmat = nc.dram_tensor("gmat", [D, H * D], f32r, kind="ExternalInput")
    wvall = nc.dram_tensor("wvall", [D, H * D], f32r, kind="ExternalInput")
    wo = nc.dram_tensor("wo", [H, D, D], f32r, kind="ExternalInput")
    w1 = nc.dram_tensor("w1", [D, 48], f32r, kind="ExternalInput")
    w2 = nc.dram_tensor("w2", [48, D], f32r, kind="ExternalInput")
    b1c = nc.dram_tensor("b1c", [48, 1], f32, kind="ExternalInput")
    b2c = nc.dram_tensor("b2c", [D, 1], f32, kind="ExternalInput")
    xout = nc.dram_tensor("xout", [TOK, D], f32, kind="ExternalOutput")

    def col0(b):  # first data col of batch b in padded tile space
        return 3 + b * STRIDE

    with tile.TileContext(nc) as tc:
        with tc.tile_pool(name="big", bufs=1) as big, \
             tc.tile_pool(name="wts", bufs=1) as wts, \
             tc.tile_pool(name="io", bufs=3) as iop, \
             tc.tile_pool(name="work", bufs=2) as work, \
             tc.tile_pool(name="sm", bufs=2) as sm, \
             tc.tile_pool(name="cs", bufs=2) as csp, \
             tc.tile_pool(name="psc", bufs=1, space="PSUM") as psc, \
             tc.tile_pool(name="psg", bufs=2, space="PSUM") as psg:

            # ---- persistent SBUF state ----
            x = big.tile([128, TILEW], f32r, tag="x")
            h = big.tile([128, TILEW], f32r, tag="h")
            sq = big.tile([128, PADW], f32r, tag="sq")

            # ---- weights/constants to SBUF ----
            pesb = wts.tile([D, S], f32r, tag="pe")
            nc.sync.dma_start(out=pesb, in_=peT[:, :])
            eyesb = wts.tile([128, 128], f32r, tag="eye")
            nc.sync.dma_start(out=eyesb, in_=eye[:, :])
            onesb = wts.tile([128, 128], f32r, tag="ones")
            nc.sync.dma_start(out=onesb, in_=ones[:, :])
            ejsb = wts.tile([D, NCH, NCH], f32r, tag="ej")
            nc.sync.dma_start(out=ejsb, in_=ejst.rearrange("j d c -> d j c"))
            bselsb = wts.tile([NCH, NCH, D], f32r, tag="bsel")
            nc.sync.dma_start(out=bselsb, in_=bsel.rearrange("j p d -> p j d"))
            mksb = wts.tile([D, L, KW, D], f32r, tag="mk")
            nc.sync.dma_start(out=mksb, in_=mk.rearrange("l k d c -> d l k c"))
            cbsb = wts.tile([D, L], f32, tag="cb")
            nc.sync.dma_start(out=cbsb, in_=cbias[:, :])
            gsb = wts.tile([D, H, D], f32r, tag="g")
            nc.sync.dma_start(out=gsb, in_=gmat.rearrange("d (h e) -> d h e", h=H))
            wvsb = wts.tile([D, H * D], f32r, tag="wv")
            nc.sync.dma_start(out=wvsb, in_=wvall[:, :])
            wosb = wts.tile([D, H, D], f32r, tag="wo")
            nc.sync.dma_start(out=wosb, in_=wo.rearrange("h d c -> d h c"))
            w1sb = wts.tile([D, 48], f32r, tag="w1")
            nc.sync.dma_start(out=w1sb, in_=w1[:, :])
            w2sb = wts.tile([48, D], f32r, tag="w2")
            nc.sync.dma_start(out=w2sb, in_=w2[:, :])
            b1sb = wts.tile([48, 1], f32, tag="b1")
            nc.sync.dma_start(out=b1sb, in_=b1c[:, :])
            b2sb = wts.tile([D, 1], f32, tag="b2")
            nc.sync.dma_start(out=b2sb, in_=b2c[:, :])
            epssb = wts.tile([128, 1], f32, tag="eps")
            nc.vector.memset(epssb, 1e-5)
            zf32 = wts.tile([128, 512], f32, tag="zf")
            nc.vector.memset(zf32, 0.0)

            def zero_guards(dst):
                nc.vector.tensor_copy(out=dst[:D, 0:3], in_=zf32[:D, 0:3])
                nc.vector.tensor_copy(
                    out=dst[:D, 3 + (BL - 1) * STRIDE + S:TILEW],
                    in_=zf32[:D, 0:TILEW - (3 + (BL - 1) * STRIDE + S)])
                gap = dst[:D, 3 + S: 3 + S + (BL - 1) * STRIDE].rearrange(
                    "d (b st) -> d b st", st=STRIDE)[:, :, :6]
                nc.vector.tensor_copy(
                    out=gap,
                    in_=zf32[:D, 0:(BL - 1) * 6].rearrange(
                        "d (b s) -> d b s", s=6))

            # zero x AND h guards once (LN writes x*rstd: zero stays zero)
            zero_guards(x)
            zero_guards(h)
            # load input transposed, *sqrt(96), +pe
            xin_t = xin.rearrange("(n p) d -> n p d", p=128)
            for j in range(TOK // 128):
                b, part = j // 3, j % 3
                tin = iop.tile([128, D], f32r, tag="tin")
                nc.sync.dma_start(out=tin, in_=xin_t[j, :, :])
                pt = psg.tile([D, 128], f32r, tag="g")
                nc.tensor.transpose(pt, tin, eyesb[:, :])
                c0 = col0(b) + 128 * part
                nc.vector.scalar_tensor_tensor(
                    out=x[:D, c0:c0 + 128], in0=pt, scalar=SQ96,
                    in1=pesb[:, 128 * part:128 * (part + 1)],
                    op0=ALU.mult, op1=ALU.add)

            # ---------------- helpers ----------------
            def layernorm(dst, center=False, guards=False):
                """dst[:D, data cols] = x * rstd  (mean folded into the
                column-centered consumer weights; g/b folded likewise).
                center=True subtracts the mean on device instead — used for
                attention, where f32r rounding of the uncentered mean
                component adds score noise that exp() amplifies."""
                nc.scalar.activation(
                    out=sq[:D, :], in_=x[:D, 3:3 + PADW], func=AF.Square)
                s1 = psc.tile([NCH, CHW], f32, tag="a0")
                s2 = psc.tile([NCH, CHW], f32, tag="a1")
                for j in range(NCH):
                    xc = x[:D, 3 + j * CHW: 3 + (j + 1) * CHW]
                    sc = sq[:D, j * CHW:(j + 1) * CHW]
                    nc.tensor.matmul(s1, ejsb[:, j, :], xc,
                                     start=(j == 0), stop=(j == NCH - 1))
                    nc.tensor.matmul(s2, ejsb[:, j, :], sc,
                                     start=(j == 0), stop=(j == NCH - 1))
                mu = sm.tile([NCH, CHW], f32, tag="mu")
                e2 = sm.tile([NCH, CHW], f32, tag="e2")
                nc.vector.tensor_scalar(out=mu, in0=s1, scalar1=1.0 / D,
                                        scalar2=None, op0=ALU.mult)
                nc.vector.tensor_scalar(out=e2, in0=s2, scalar1=1.0 / D,
                                        scalar2=1e-5, op0=ALU.mult,
                                        op1=ALU.add)
                var = sm.tile([NCH, CHW], f32, tag="var")
                nc.vector.tensor_tensor(out=var, in0=mu, in1=mu, op=ALU.mult)
                nc.vector.tensor_tensor(out=var, in0=e2, in1=var,
                                        op=ALU.subtract)
                rq = sm.tile([NCH, CHW], f32, tag="rq")
                nc.vector.reciprocal_approx_fast(out=rq[:, :], in_=var[:, :])
                rr = sm.tile([NCH, CHW], f32r, tag="rr")
                nc.scalar.activation(out=rr, in_=rq, func=AF.Sqrt)
                if center:
                    mr = sm.tile([NCH, CHW], f32r, tag="mr")
                    nc.vector.tensor_tensor(out=mr, in0=mu, in1=rr,
                                            op=ALU.mult)
                for j in range(NCH):
                    rbc = psg.tile([D, CHW], f32, tag="g")
                    nc.tensor.matmul(rbc, bselsb[:, j, :], rr,
                                     start=True, stop=True)
                    c0 = 3 + j * CHW
                    nc.vector.tensor_tensor(out=dst[:D, c0:c0 + CHW],
                                            in0=x[:D, c0:c0 + CHW], in1=rbc,
                                            op=ALU.mult)
                    if center:
                        mbc = psg.tile([D, CHW], f32, tag="g")
                        nc.tensor.matmul(mbc, bselsb[:, j, :], mr,
                                         start=True, stop=True)
                        nc.vector.tensor_tensor(out=dst[:D, c0:c0 + CHW],
                                                in0=dst[:D, c0:c0 + CHW],
                                                in1=mbc, op=ALU.subtract)
                if guards:
                    # x's guard gaps are polluted by the conv residual adds;
                    # conv inputs must see zeros across batch boundaries
                    zero_guards(dst)

            # ---------------- conv blocks ----------------
            for li in range(L):
                layernorm(h, guards=True)
                for j in range(NCH):
                    pc = psg.tile([D, CHW], f32, tag="g")
                    for k in range(KW):
                        rhs = h[:D, j * CHW + k: j * CHW + k + CHW]
                        nc.tensor.matmul(pc, mksb[:, li, k, :], rhs,
                                         start=(k == 0), stop=(k == KW - 1))
                    cs = csp.tile([D, CHW], f32r, tag="cs")
                    nc.scalar.activation(out=cs, in_=pc, func=AF.Relu,
                                         bias=cbsb[:, li:li + 1])
                    c0 = 3 + j * CHW
                    nc.vector.tensor_tensor(out=x[:D, c0:c0 + CHW],
                                            in0=x[:D, c0:c0 + CHW], in1=cs,
                                            op=ALU.add)

            # ---------------- attention ----------------
            layernorm(h)
            for b in range(BL):
                hb = h[:D, col0(b):col0(b) + S]
                # V projections, one 97-col block per head ([V_h | 1])
                vt = work.tile([128, 3, H, D + 1], f32r, tag="vt")
                nc.vector.tensor_copy(
                    out=vt[:, :, :, D:D + 1],
                    in_=onesb[:, 0:3 * H].rearrange(
                        "p (a h o) -> p a h o", a=3, h=H))
                for c in range(3):
                    pv = psg.tile([128, H * D], f32, tag="g")
                    nc.tensor.matmul(
                        pv, h[:D, col0(b) + 128 * c: col0(b) + 128 * (c + 1)],
                        wvsb, start=True, stop=True)
                    nc.vector.tensor_copy(
                        out=vt[:, c, :, 0:D],
                        in_=pv.rearrange("p (h d) -> p h d", h=H))
                ut = work.tile([D, H, S], f32r, tag="ut")
                for hh in range(H):
                    pu = psg.tile([D, S], f32, tag="g")
                    nc.tensor.matmul(pu, gsb[:, hh, :], hb,
                                     start=True, stop=True)
                    nc.vector.tensor_copy(out=ut[:, hh, :], in_=pu)
                cat = work.tile([D, H, S], f32r, tag="cat")
                for hh in range(H):
                    # explicit tag double-buffering: heads h and h+1 own
                    # disjoint score banks so their pipelines overlap
                    ps = [psc.tile([128, 512], f32, tag=f"a{(hh % 2) * 3 + c}",
                                   name=f"sc{b}_{hh}_{c}")
                          for c in range(3)]
                    scp = work.tile([128, 3, S], f32, tag="scp")
                    for c in range(3):
                        lhsT = h[:D, col0(b) + 128 * c: col0(b) + 128 * (c + 1)]
                        # stop is sim-only; group continues via the -max
                        # accumulate below (skip_group_check)
                        nc.tensor.matmul(ps[c][:, :S], lhsT, ut[:, hh, :],
                                         start=True, stop=True)
                        # raw scores to SBUF (scalar engine) for the gpsimd max
                        nc.scalar.activation(out=scp[:, c, :],
                                             in_=ps[c][:, :S], func=AF.Copy)
                    # true per-query max over all 3 key chunks in one gpsimd op
                    mx = sm.tile([128, 3, S], f32, tag="mx")
                    nc.gpsimd.partition_all_reduce(
                        mx[:, :, :], scp[:, :, :], channels=128,
                        reduce_op=bass_isa.ReduceOp.max)
                    nc.vector.tensor_tensor(out=mx[0:1, 0, :],
                                            in0=mx[0:1, 0, :],
                                            in1=mx[0:1, 1, :], op=ALU.max)
                    nc.vector.tensor_tensor(out=mx[0:1, 0, :],
                                            in0=mx[0:1, 0, :],
                                            in1=mx[0:1, 2, :], op=ALU.max)
                    mneg = sm.tile([1, S], f32r, tag="mneg")
                    nc.vector.tensor_scalar(out=mneg, in0=mx[0:1, 0, :],
                                            scalar1=-1.0, scalar2=None,
                                            op0=ALU.mult)
                    et = work.tile([128, 3, S], f32r, tag="et")
                    pctx = psg.tile([97, S], f32, tag="g")
                    for c in range(3):
                        nc.tensor.matmul(ps[c][:, :S], onesb[0:1, :],
                                         mneg, start=False, stop=True,
                                         skip_group_check=True)
                        nc.scalar.activation(out=et[:, c, :], in_=ps[c][:, :S],
                                             func=AF.Exp)
                        nc.tensor.matmul(pctx, vt[:, c, hh, :], et[:, c, :],
                                         start=(c == 0), stop=(c == 2))
                    # row 96 of pctx = Z; 1/Z broadcast to the 96 ctx rows.
                    # (stage Z via SBUF: reciprocal_approx_fast's bit-trick
                    # reads garbage from PSUM directly on HW)
                    zrow = sm.tile([1, S], f32, tag="zrow")
                    nc.scalar.activation(out=zrow, in_=pctx[96:97, :],
                                         func=AF.Copy)
                    zr = sm.tile([1, S], f32, tag="zr")
                    nc.vector.reciprocal_approx_fast(out=zr, in_=zrow[:, :])
                    zbc = sm.tile([D, S], f32, tag="zbc")
                    nc.gpsimd.partition_broadcast(zbc, zr, channels=D)
                    nc.vector.tensor_tensor(out=cat[:, hh, :],
                                            in0=pctx[0:D, :], in1=zbc,
                                            op=ALU.mult)
                pwo = psg.tile([D, S], f32, tag="g")
                for hh in range(H):
                    nc.tensor.matmul(pwo, wosb[:, hh, :], cat[:, hh, :],
                                     start=(hh == 0), stop=(hh == H - 1))
                nc.vector.tensor_tensor(out=x[:D, col0(b):col0(b) + S],
                                        in0=x[:D, col0(b):col0(b) + S],
                                        in1=pwo, op=ALU.add)

            # ---------------- FFN ----------------
            layernorm(h)
            for j in range(NCH):
                hc = h[:D, 3 + j * CHW: 3 + (j + 1) * CHW]
                p1 = psg.tile([48, CHW], f32, tag="g")
                nc.tensor.matmul(p1, w1sb, hc, start=True, stop=True)
                ss = csp.tile([48, CHW], f32r, tag="ss")
                nc.scalar.activation(out=ss, in_=p1, func=AF.Sigmoid,
                                     bias=b1sb)
                p2 = psg.tile([D, CHW], f32, tag="g")
                nc.tensor.matmul(p2, w2sb, ss, start=True, stop=True)
                c0 = 3 + j * CHW
                nc.vector.scalar_tensor_tensor(
                    out=x[:D, c0:c0 + CHW], in0=p2, scalar=b2sb,
                    in1=x[:D, c0:c0 + CHW], op0=ALU.add, op1=ALU.add)

            # ---------------- store output (transpose back) ----------------
            xout_t = xout.rearrange("(n p) d -> n p d", p=128)
            for j in range(TOK // 128):
                b, part = j // 3, j % 3
                c0 = col0(b) + 128 * part
                po = psg.tile([128, D], f32r, tag="g")
                nc.tensor.transpose(po, x[:D, c0:c0 + 128], eyesb[:D, :D])
                ot = iop.tile([128, D], f32, tag="ot")
                nc.vector.tensor_copy(out=ot, in_=po)
                nc.sync.dma_start(out=xout_t[j, :, :], in_=ot)

    nc.compile()
    return nc


def _host_prep(inputs):
    """Host-side weight preprocessing -> per-NEFF input dict (shared part)."""
    f = np.float32
    conv_dw = np.asarray(inputs["conv_dw"], f)
    conv_dw_b = np.asarray(inputs["conv_dw_b"], f)
    conv_pw = np.asarray(inputs["conv_pw"], f)
    conv_pw_b = np.asarray(inputs["conv_pw_b"], f)
    WQ = np.asarray(inputs["WQ"], f)
    WK = np.asarray(inputs["WK"], f)
    WV = np.asarray(inputs["WV"], f)
    WO = np.asarray(inputs["WO"], f)
    ffn_w1 = np.asarray(inputs["ffn_w1"], f)
    ffn_b1 = np.asarray(inputs["ffn_b1"], f)
    ffn_w2 = np.asarray(inputs["ffn_w2"], f)
    ffn_b2 = np.asarray(inputs["ffn_b2"], f)
    ln_g = np.asarray(inputs["ln_g"], f)
    ln_b = np.asarray(inputs["ln_b"], f)

    # positional encoding (faithful to reference)
    pos = np.arange(S, dtype=f)[:, None]
    i = np.arange(0, D, 2, dtype=f)
    pe = np.zeros((S, D), f)
    pe[:, 0::2] = np.sin(pos / 10000.0 ** (2.0 * i / D))
    pe[:, 1::2] = np.cos(pos / 10000.0 ** (2.0 * (i + 1.0) / D))

    # LN mean subtraction folded into consumers: column-center every matrix
    # that left-multiplies the LN output (M'^T x == M^T (x - mean(x))).
    mk = np.zeros((L, KW, D, D), f)
    cbias = np.zeros((L, D), f)
    for li in range(L):
        g, bb = ln_g[li], ln_b[li]
        pwT = conv_pw[li][:, :, 0].T          # [d_in, c_out]
        for k in range(KW):
            m = pwT * (conv_dw[li][:, 0, k] * g)[:, None]
            mk[li, k] = m - m.mean(axis=0, keepdims=True)
        t = bb * conv_dw[li][:, 0, :].sum(-1) + conv_dw_b[li]
        cbias[li] = conv_pw_b[li] + conv_pw[li][:, :, 0] @ t

    g4 = ln_g[L]
    blocks = []
    for hh in range(H):
        Gh = (WQ[hh] @ WK[hh].T) * np.outer(g4, g4) * f(SQ96)
        Gh = Gh - Gh.mean(axis=0, keepdims=True) \
            - Gh.mean(axis=1, keepdims=True) + Gh.mean()
        blocks.append(Gh)
    gmat = np.concatenate(blocks, axis=1)      # [d, H*d'], double-centered
    wvall = np.concatenate([g4[:, None] * WV[hh] for hh in range(H)], axis=1)
    wvall = wvall - wvall.mean(axis=0, keepdims=True)

    g5 = ln_g[L + 1]
    w1f = g5[:, None] * ffn_w1
    w1f = w1f - w1f.mean(axis=0, keepdims=True)
    b1f = ffn_b1 + ffn_w1.T @ ln_b[L + 1]

    ejst = np.zeros((NCH, D, NCH), f)
    bsel = np.zeros((NCH, NCH, D), f)
    for j in range(NCH):
        ejst[j, :, j] = 1.0
        bsel[j, j, :] = 1.0

    return {
        "peT": np.ascontiguousarray(pe.T),
        "eye": np.eye(128, dtype=f),
        "ones": np.ones((128, 128), f),
        "ejst": ejst,
        "bsel": bsel,
        "mk": mk,
        "cbias": np.ascontiguousarray(cbias.T),
        "gmat": gmat,
        "wvall": wvall,
        "wo": np.ascontiguousarray(WO.reshape(H, D, D)),
        "w1": w1f,
        "w2": ffn_w2,
        "b1c": b1f[:, None],
        "b2c": ffn_b2[:, None],
    }


def kernel(**inputs) -> np.ndarray:
    from concourse.bass_utils import run_bass_kernel_spmd

    if "nc" not in _cache:
        _cache["nc"] = _build_module()
    nc = _cache["nc"]

    shared = _host_prep(inputs)
    xfull = np.asarray(inputs["input"], np.float32)  # [B, S, D]
    in_maps = []
    for c in range(NCORES):
        m = dict(shared)
        m["xin"] = np.ascontiguousarray(
            xfull[c * BL:(c + 1) * BL].reshape(TOK, D))
        in_maps.append(m)

    res = run_bass_kernel_spmd(nc, in_maps, core_ids=list(range(NCORES)))
    out = np.empty((B, S, D), np.float32)
    for c in range(NCORES):
        out[c * BL:(c + 1) * BL] = res.results[c]["xout"].reshape(BL, S, D)
    return out


# revision 32
# speedup vs baseline: 1.1522x; 1.1522x over previous
"""Trainium2 Bass kernel for nn_EmbeddingEncoder (dense transformer encoder).

Strategy (8 cores, data-parallel over batch, 16 batches/core):
- Canonical activation layout: channels-first [96, tokens] in SBUF, with
  6-col zero guards between batches (+3 outer) so the depthwise conv's
  shifted windows never cross batch boundaries.
- All matmuls in float32r (TF32-like, 1 cyc/row at N>=256).
- LN with zero-mean weight folding: every consumer weight matrix M of the
  LN output is column-centered on the host (M' = P M, P = I - 11^T/D), so
  mean subtraction never happens on device; LN = x * rstd_broadcast only.
  Stats via ones-column matmuls -> [13,480] compact tiles; rstd broadcast
  back via K=13 selector matmuls; reciprocal via fast-approx DVE op.
- Conv block: depthwise+pointwise fused into 7 per-tap [96,96] matrices
  M_k = pw^T * dw_k (host-precomputed, column-centered), 7 accumulating
  matmuls per chunk.
- Attention: scores computed transposed ([k,q]); true per-query max via
  gpsimd partition_all_reduce(max) on the score PSUM chunks (no LSE bound,
  no Ln, single Exp pass -> no activation-table thrash); -max applied into
  the open score PSUM group by rank-1 ones matmuls; softmax denominator Z
  piggybacks as a 97th all-ones column on the V matrix so the ctx matmul
  computes it for free; 1/Z via fast-approx reciprocal, broadcast to the
  96 ctx rows via gpsimd partition_broadcast.
"""
import sys
import math

sys.path.insert(0, "/opt/trn_rl_repo")

import numpy as np

B, S, D, H, KW, L = 128, 384, 96, 4, 7, 4
NCORES = 8
BL = B // NCORES            # 16 batches per core
TOK = BL * S                # 6144 tokens per core
STRIDE = S + 6              # 390: batch stride in padded layout
PADW = 3 + BL * STRIDE - 6 + 3  # data width 6240
TILEW = PADW + 6            # 6246 incl 3-col outer guards both sides
NCH = 13                    # LN/conv/ffn chunking
CHW = 480                   # 13*480 = 6240
SQ96 = math.sqrt(96.0)

_cache = {}


def _build_module():
    import concourse.bass as bass
    import concourse.bacc as bacc
    import concourse.mybir as mybir
    import concourse.tile as tile
    import concourse.bass_isa as bass_isa

    f32 = mybir.dt.float32
    f32r = mybir.dt.float32r
    AF = mybir.ActivationFunctionType
    ALU = mybir.AluOpType

    nc = bacc.Bacc("TRN2", target_bir_lowering=False)

    # ---- DRAM tensors ----
    xin = nc.dram_tensor("xin", [TOK, D], f32r, kind="ExternalInput")
    peT = nc.dram_tensor("peT", [D, S], f32r, kind="ExternalInput")
    eye = nc.dram_tensor("eye", [128, 128], f32r, kind="ExternalInput")
    ones = nc.dram_tensor("ones", [128, 128], f32r, kind="ExternalInput")
    ejst = nc.dram_tensor("ejst", [NCH, D, NCH], f32r, kind="ExternalInput")
    bsel = nc.dram_tensor("bsel", [NCH, NCH, D], f32r, kind="ExternalInput")
    mk = nc.dram_tensor("mk", [L, KW, D, D], f32r, kind="ExternalInput")
    cbias = nc.dram_tensor("cbias", [D, L], f32, kind="ExternalInput")
    gmat = nc.dram_tensor("gmat", [D, H * D], f32r, kind="ExternalInput")
    wvall = nc.dram_tensor("wvall", [D, H * D], f32r, kind="ExternalInput")
    wo = nc.dram_tensor("wo", [H, D, D], f32r, kind="ExternalInput")
    w1 = nc.dram_tensor("w1", [D, 48], f32r, kind="ExternalInput")
    w2 = nc.dram_tensor("w2", [48, D], f32r, kind="ExternalInput")
    b1c = nc.dram_tensor("b1c", [48, 1], f32, kind="ExternalInput")
    b2c = nc.dram_tensor("b2c", [D, 1], f32, kind="ExternalInput")
    xout = nc.dram_tensor("xout", [TOK, D], f32, kind="ExternalOutput")

    def col0(b):  # first data col of batch b in padded tile space
        return 3 + b * STRIDE

    with tile.TileContext(nc) as tc:
        with tc.tile_pool(name="big", bufs=1) as big, \
             tc.tile_pool(name="wts", bufs=1) as wts, \
             tc.tile_pool(name="io", bufs=3) as iop, \
             tc.tile_pool(name="work", bufs=2) as work, \
             tc.tile_pool(name="sm", bufs=2) as sm, \
             tc.tile_pool(name="cs", bufs=2) as csp, \
             tc.tile_pool(name="psc", bufs=1, space="PSUM") as psc, \
             tc.tile_pool(name="psg", bufs=2, space="PSUM") as psg:

            # ---- persistent SBUF state ----
            x = big.tile([128, TILEW], f32r, tag="x")
            h = big.tile([128, TILEW], f32r, tag="h")
            sq = big.tile([128, PADW], f32r, tag="sq")

            # ---- weights/constants to SBUF ----
            pesb = wts.tile([D, S], f32r, tag="pe")
            nc.sync.dma_start(out=pesb, in_=peT[:, :])
            eyesb = wts.tile([128, 128], f32r, tag="eye")
            nc.sync.dma_start(out=eyesb, in_=eye[:, :])
            onesb = wts.tile([128, 128], f32r, tag="ones")
            nc.sync.dma_start(out=onesb, in_=ones[:, :])
            ejsb = wts.tile([D, NCH, NCH], f32r, tag="ej")
            nc.sync.dma_start(out=ejsb, in_=ejst.rearrange("j d c -> d j c"))
            bselsb = wts.tile([NCH, NCH, D], f32r, tag="bsel")
            nc.sync.dma_start(out=bselsb, in_=bsel.rearrange("j p d -> p j d"))
            mksb = wts.tile([D, L, KW, D], f32r, tag="mk")
            nc.sync.dma_start(out=mksb, in_=mk.rearrange("l k d c -> d l k c"))
            cbsb = wts.tile([D, L], f32, tag="cb")
            nc.sync.dma_start(out=cbsb, in_=cbias[:, :])
            gsb = wts.tile([D, H, D], f32r, tag="g")
            nc.sync.dma_start(out=gsb, in_=gmat.rearrange("d (h e) -> d h e", h=H))
            wvsb = wts.tile([D, H * D], f32r, tag="wv")
            nc.sync.dma_start(out=wvsb, in_=wvall[:, :])
            wosb = wts.tile([D, H, D], f32r, tag="wo")
            nc.sync.dma_start(out=wosb, in_=wo.rearrange("h d c -> d h c"))
            w1sb = wts.tile([D, 48], f32r, tag="w1")
            nc.sync.dma_start(out=w1sb, in_=w1[:, :])
            w2sb = wts.tile([48, D], f32r, tag="w2")
            nc.sync.dma_start(out=w2sb, in_=w2[:, :])
            b1sb = wts.tile([48, 1], f32, tag="b1")
            nc.sync.dma_start(out=b1sb, in_=b1c[:, :])
            b2sb = wts.tile([D, 1], f32, tag="b2")
            nc.sync.dma_start(out=b2sb, in_=b2c[:, :])
            epssb = wts.tile([128, 1], f32, tag="eps")
            nc.vector.memset(epssb, 1e-5)
            zf32 = wts.tile([128, 512], f32, tag="zf")
            nc.vector.memset(zf32, 0.0)

            def zero_guards(dst):
                nc.vector.tensor_copy(out=dst[:D, 0:3], in_=zf32[:D, 0:3])
                nc.vector.tensor_copy(
                    out=dst[:D, 3 + (BL - 1) * STRIDE + S:TILEW],
                    in_=zf32[:D, 0:TILEW - (3 + (BL - 1) * STRIDE + S)])
                gap = dst[:D, 3 + S: 3 + S + (BL - 1) * STRIDE].rearrange(
                    "d (b st) -> d b st", st=STRIDE)[:, :, :6]
                nc.vector.tensor_copy(
                    out=gap,
                    in_=zf32[:D, 0:(BL - 1) * 6].rearrange(
                        "d (b s) -> d b s", s=6))

            # zero x AND h guards once (LN writes x*rstd: zero stays zero)
            zero_guards(x)
            zero_guards(h)
            # load input transposed, *sqrt(96), +pe
            xin_t = xin.rearrange("(n p) d -> n p d", p=128)
            for j in range(TOK // 128):
                b, part = j // 3, j % 3
                tin = iop.tile([128, D], f32r, tag="tin")
                nc.sync.dma_start(out=tin, in_=xin_t[j, :, :])
                pt = psg.tile([D, 128], f32r, tag="g")
                nc.tensor.transpose(pt, tin, eyesb[:, :])
                c0 = col0(b) + 128 * part
                nc.vector.scalar_tensor_tensor(
                    out=x[:D, c0:c0 + 128], in0=pt, scalar=SQ96,
                    in1=pesb[:, 128 * part:128 * (part + 1)],
                    op0=ALU.mult, op1=ALU.add)

            # ---------------- helpers ----------------
            def layernorm(dst, center=False, guards=False):
                """dst[:D, data cols] = x * rstd  (mean folded into the
                column-centered consumer weights; g/b folded likewise).
                center=True subtracts the mean on device instead — used for
                attention, where f32r rounding of the uncentered mean
                component adds score noise that exp() amplifies."""
                nc.scalar.activation(
                    out=sq[:D, :], in_=x[:D, 3:3 + PADW], func=AF.Square)
                s1 = psc.tile([NCH, CHW], f32, tag="a0")
                s2 = psc.tile([NCH, CHW], f32, tag="a1")
                for j in range(NCH):
                    xc = x[:D, 3 + j * CHW: 3 + (j + 1) * CHW]
                    sc = sq[:D, j * CHW:(j + 1) * CHW]
                    nc.tensor.matmul(s1, ejsb[:, j, :], xc,
                                     start=(j == 0), stop=(j == NCH - 1))
                    nc.tensor.matmul(s2, ejsb[:, j, :], sc,
                                     start=(j == 0), stop=(j == NCH - 1))
                mu = sm.tile([NCH, CHW], f32, tag="mu")
                e2 = sm.tile([NCH, CHW], f32, tag="e2")
                nc.vector.tensor_scalar(out=mu, in0=s1, scalar1=1.0 / D,
                                        scalar2=None, op0=ALU.mult)
                nc.vector.tensor_scalar(out=e2, in0=s2, scalar1=1.0 / D,
                                        scalar2=1e-5, op0=ALU.mult,
                                        op1=ALU.add)
                var = sm.tile([NCH, CHW], f32, tag="var")
                nc.vector.tensor_tensor(out=var, in0=mu, in1=mu, op=ALU.mult)
                nc.vector.tensor_tensor(out=var, in0=e2, in1=var,
                                        op=ALU.subtract)
                rq = sm.tile([NCH, CHW], f32, tag="rq")
                nc.vector.reciprocal_approx_fast(out=rq[:, :], in_=var[:, :])
                rr = sm.tile([NCH, CHW], f32r, tag="rr")
                nc.scalar.activation(out=rr, in_=rq, func=AF.Sqrt)
                if center:
                    mr = sm.tile([NCH, CHW], f32r, tag="mr")
                    nc.vector.tensor_tensor(out=mr, in0=mu, in1=rr,
                                            op=ALU.mult)
                for j in range(NCH):
                    rbc = psg.tile([D, CHW], f32, tag="g")
                    nc.tensor.matmul(rbc, bselsb[:, j, :], rr,
                                     start=True, stop=True)
                    c0 = 3 + j * CHW
                    nc.vector.tensor_tensor(out=dst[:D, c0:c0 + CHW],
                                            in0=x[:D, c0:c0 + CHW], in1=rbc,
                                            op=ALU.mult)
                    if center:
                        mbc = psg.tile([D, CHW], f32, tag="g")
                        nc.tensor.matmul(mbc, bselsb[:, j, :], mr,
                                         start=True, stop=True)
                        nc.vector.tensor_tensor(out=dst[:D, c0:c0 + CHW],
                                                in0=dst[:D, c0:c0 + CHW],
                                                in1=mbc, op=ALU.subtract)
                if guards:
                    # x's guard gaps are polluted by the conv residual adds;
                    # conv inputs must see zeros across batch boundaries
                    zero_guards(dst)

            # ---------------- conv blocks ----------------
            for li in range(L):
                layernorm(h, guards=True)
                for j in range(NCH):
                    pc = psg.tile([D, CHW], f32, tag="g")
                    for k in range(KW):
                        rhs = h[:D, j * CHW + k: j * CHW + k + CHW]
                        nc.tensor.matmul(pc, mksb[:, li, k, :], rhs,
                                         start=(k == 0), stop=(k == KW - 1))
                    cs = csp.tile([D, CHW], f32r, tag="cs")
                    nc.scalar.activation(out=cs, in_=pc, func=AF.Relu,
                                         bias=cbsb[:, li:li + 1])
                    c0 = 3 + j * CHW
                    nc.vector.tensor_tensor(out=x[:D, c0:c0 + CHW],
                                            in0=x[:D, c0:c0 + CHW], in1=cs,
                                            op=ALU.add)

            # ---------------- attention ----------------
            # software-pipelined: the engines are in-order, so emit head
            # h+1's score matmuls BEFORE head h's dependency-stalled tail,
            # and batch b+1's projections before batch b's WO projection.
            layernorm(h)

            def attn_scores(b, hh, ut):
                """Scores for (b, hh) + SBUF staging; returns live tiles."""
                ps = [psc.tile([128, 512], f32, tag=f"a{(hh % 2) * 3 + c}",
                               name=f"sc{b}_{hh}_{c}")
                      for c in range(3)]
                scp = work.tile([128, 3, S], f32, tag="scp",
                                name=f"scp{b}_{hh}")
                for c in range(3):
                    lhsT = h[:D, col0(b) + 128 * c: col0(b) + 128 * (c + 1)]
                    # stop is sim-only; group continues via the -max
                    # accumulate below (skip_group_check)
                    nc.tensor.matmul(ps[c][:, :S], lhsT, ut[:, hh, :],
                                     start=True, stop=True)
                    # raw scores to SBUF (scalar engine) for the gpsimd max
                    nc.scalar.activation(out=scp[:, c, :],
                                         in_=ps[c][:, :S], func=AF.Copy)
                # per-query max over all 3 key chunks in one gpsimd op
                mx = sm.tile([128, 3, S], f32, tag="mx", name=f"mx{b}_{hh}")
                nc.gpsimd.partition_all_reduce(
                    mx[:, :, :], scp[:, :, :], channels=128,
                    reduce_op=bass_isa.ReduceOp.max)
                return ps, mx

            def attn_tail(b, hh, ps, mx, vt, cat):
                nc.vector.tensor_tensor(out=mx[0:1, 0, :], in0=mx[0:1, 0, :],
                                        in1=mx[0:1, 1, :], op=ALU.max)
                nc.vector.tensor_tensor(out=mx[0:1, 0, :], in0=mx[0:1, 0, :],
                                        in1=mx[0:1, 2, :], op=ALU.max)
                mneg = sm.tile([1, S], f32r, tag="mneg", name=f"mn{b}_{hh}")
                nc.vector.tensor_scalar(out=mneg, in0=mx[0:1, 0, :],
                                        scalar1=-1.0, scalar2=None,
                                        op0=ALU.mult)
                et = work.tile([128, 3, S], f32r, tag="et", name=f"et{b}_{hh}")
                pctx = psg.tile([97, S], f32, tag="g", name=f"cx{b}_{hh}")
                for c in range(3):
                    nc.tensor.matmul(ps[c][:, :S], onesb[0:1, :],
                                     mneg, start=False, stop=True,
                                     skip_group_check=True)
                    nc.scalar.activation(out=et[:, c, :], in_=ps[c][:, :S],
                                         func=AF.Exp)
                    nc.tensor.matmul(pctx, vt[:, c, hh, :], et[:, c, :],
                                     start=(c == 0), stop=(c == 2))
                # row 96 of pctx = Z; 1/Z broadcast to the 96 ctx rows.
                # (stage Z via SBUF: reciprocal_approx_fast's bit-trick
                # reads garbage from PSUM directly on HW)
                zrow = sm.tile([1, S], f32, tag="zrow", name=f"zw{b}_{hh}")
                nc.scalar.activation(out=zrow, in_=pctx[96:97, :],
                                     func=AF.Copy)
                zr = sm.tile([1, S], f32, tag="zr", name=f"zi{b}_{hh}")
                nc.vector.reciprocal_approx_fast(out=zr, in_=zrow[:, :])
                zbc = sm.tile([D, S], f32, tag="zbc", name=f"zb{b}_{hh}")
                nc.gpsimd.partition_broadcast(zbc, zr, channels=D)
                nc.vector.tensor_tensor(out=cat[:, hh, :],
                                        in0=pctx[0:D, :], in1=zbc,
                                        op=ALU.mult)

            def attn_proj(b):
                """V/Q projections for batch b; returns vt, ut, cat tiles."""
                hb = h[:D, col0(b):col0(b) + S]
                vt = work.tile([128, 3, H, D + 1], f32r, tag="vt",
                               name=f"vt{b}")
                nc.vector.tensor_copy(
                    out=vt[:, :, :, D:D + 1],
                    in_=onesb[:, 0:3 * H].rearrange(
                        "p (a h o) -> p a h o", a=3, h=H))
                for c in range(3):
                    pv = psg.tile([128, H * D], f32, tag="g", name=f"pv{b}_{c}")
                    nc.tensor.matmul(
                        pv, h[:D, col0(b) + 128 * c: col0(b) + 128 * (c + 1)],
                        wvsb, start=True, stop=True)
                    nc.vector.tensor_copy(
                        out=vt[:, c, :, 0:D],
                        in_=pv.rearrange("p (h d) -> p h d", h=H))
                ut = work.tile([D, H, S], f32r, tag="ut", name=f"ut{b}")
                for hh in range(H):
                    pu = psg.tile([D, S], f32, tag="g", name=f"pu{b}_{hh}")
                    nc.tensor.matmul(pu, gsb[:, hh, :], hb,
                                     start=True, stop=True)
                    nc.vector.tensor_copy(out=ut[:, hh, :], in_=pu)
                cat = work.tile([D, H, S], f32r, tag="cat", name=f"cat{b}",
                                bufs=3)
                return vt, ut, cat

            def attn_wo(b, cat):
                pwo = psg.tile([D, S], f32, tag="g", name=f"pwo{b}")
                for hh in range(H):
                    nc.tensor.matmul(pwo, wosb[:, hh, :], cat[:, hh, :],
                                     start=(hh == 0), stop=(hh == H - 1))
                nc.vector.tensor_tensor(out=x[:D, col0(b):col0(b) + S],
                                        in0=x[:D, col0(b):col0(b) + S],
                                        in1=pwo, op=ALU.add)

            prev_cat = None
            vt, ut, cat = attn_proj(0)
            for b in range(BL):
                pend = None
                for hh in range(H):
                    cur = attn_scores(b, hh, ut)
                    if pend is not None:
                        attn_tail(b, hh - 1, *pend, vt, cat)
                    pend = cur
                # next batch's projections overlap this batch's last tails
                nxt = attn_proj(b + 1) if b + 1 < BL else None
                attn_tail(b, H - 1, *pend, vt, cat)
                if prev_cat is not None:
                    attn_wo(b - 1, prev_cat)
                prev_cat = cat
                if nxt is not None:
                    vt, ut, cat = nxt
            attn_wo(BL - 1, prev_cat)

            # ---------------- FFN ----------------
            layernorm(h)
            for j in range(NCH):
                hc = h[:D, 3 + j * CHW: 3 + (j + 1) * CHW]
                p1 = psg.tile([48, CHW], f32, tag="g")
                nc.tensor.matmul(p1, w1sb, hc, start=True, stop=True)
                ss = csp.tile([48, CHW], f32r, tag="ss")
                nc.scalar.activation(out=ss, in_=p1, func=AF.Sigmoid,
                                     bias=b1sb)
                p2 = psg.tile([D, CHW], f32, tag="g")
                nc.tensor.matmul(p2, w2sb, ss, start=True, stop=True)
                c0 = 3 + j * CHW
                nc.vector.scalar_tensor_tensor(
                    out=x[:D, c0:c0 + CHW], in0=p2, scalar=b2sb,
                    in1=x[:D, c0:c0 + CHW], op0=ALU.add, op1=ALU.add)

            # ---------------- store output (transpose back) ----------------
            xout_t = xout.rearrange("(n p) d -> n p d", p=128)
            for j in range(TOK // 128):
                b, part = j // 3, j % 3
                c0 = col0(b) + 128 * part
                po = psg.tile([128, D], f32r, tag="g")
                nc.tensor.transpose(po, x[:D, c0:c0 + 128], eyesb[:D, :D])
                ot = iop.tile([128, D], f32, tag="ot")
                nc.vector.tensor_copy(out=ot, in_=po)
                nc.sync.dma_start(out=xout_t[j, :, :], in_=ot)

    nc.compile()
    return nc


def _host_prep(inputs):
    """Host-side weight preprocessing -> per-NEFF input dict (shared part)."""
    f = np.float32
    conv_dw = np.asarray(inputs["conv_dw"], f)
    conv_dw_b = np.asarray(inputs["conv_dw_b"], f)
    conv_pw = np.asarray(inputs["conv_pw"], f)
    conv_pw_b = np.asarray(inputs["conv_pw_b"], f)
    WQ = np.asarray(inputs["WQ"], f)
    WK = np.asarray(inputs["WK"], f)
    WV = np.asarray(inputs["WV"], f)
    WO = np.asarray(inputs["WO"], f)
    ffn_w1 = np.asarray(inputs["ffn_w1"], f)
    ffn_b1 = np.asarray(inputs["ffn_b1"], f)
    ffn_w2 = np.asarray(inputs["ffn_w2"], f)
    ffn_b2 = np.asarray(inputs["ffn_b2"], f)
    ln_g = np.asarray(inputs["ln_g"], f)
    ln_b = np.asarray(inputs["ln_b"], f)

    # positional encoding (faithful to reference)
    pos = np.arange(S, dtype=f)[:, None]
    i = np.arange(0, D, 2, dtype=f)
    pe = np.zeros((S, D), f)
    pe[:, 0::2] = np.sin(pos / 10000.0 ** (2.0 * i / D))
    pe[:, 1::2] = np.cos(pos / 10000.0 ** (2.0 * (i + 1.0) / D))

    # LN mean subtraction folded into consumers: column-center every matrix
    # that left-multiplies the LN output (M'^T x == M^T (x - mean(x))).
    mk = np.zeros((L, KW, D, D), f)
    cbias = np.zeros((L, D), f)
    for li in range(L):
        g, bb = ln_g[li], ln_b[li]
        pwT = conv_pw[li][:, :, 0].T          # [d_in, c_out]
        for k in range(KW):
            m = pwT * (conv_dw[li][:, 0, k] * g)[:, None]
            mk[li, k] = m - m.mean(axis=0, keepdims=True)
        t = bb * conv_dw[li][:, 0, :].sum(-1) + conv_dw_b[li]
        cbias[li] = conv_pw_b[li] + conv_pw[li][:, :, 0] @ t

    g4 = ln_g[L]
    blocks = []
    for hh in range(H):
        Gh = (WQ[hh] @ WK[hh].T) * np.outer(g4, g4) * f(SQ96)
        Gh = Gh - Gh.mean(axis=0, keepdims=True) \
            - Gh.mean(axis=1, keepdims=True) + Gh.mean()
        blocks.append(Gh)
    gmat = np.concatenate(blocks, axis=1)      # [d, H*d'], double-centered
    wvall = np.concatenate([g4[:, None] * WV[hh] for hh in range(H)], axis=1)
    wvall = wvall - wvall.mean(axis=0, keepdims=True)

    g5 = ln_g[L + 1]
    w1f = g5[:, None] * ffn_w1
    w1f = w1f - w1f.mean(axis=0, keepdims=True)
    b1f = ffn_b1 + ffn_w1.T @ ln_b[L + 1]

    ejst = np.zeros((NCH, D, NCH), f)
    bsel = np.zeros((NCH, NCH, D), f)
    for j in range(NCH):
        ejst[j, :, j] = 1.0
        bsel[j, j, :] = 1.0

    return {
        "peT": np.ascontiguousarray(pe.T),
        "eye": np.eye(128, dtype=f),
        "ones": np.ones((128, 128), f),
        "ejst": ejst,
        "bsel": bsel,
        "mk": mk,
        "cbias": np.ascontiguousarray(cbias.T),
        "gmat": gmat,
        "wvall": wvall,
        "wo": np.ascontiguousarray(WO.reshape(H, D, D)),
        "w1": w1f,
        "w2": ffn_w2,
        "b1c": b1f[:, None],
        "b2c": ffn_b2[:, None],
    }


def kernel(**inputs) -> np.ndarray:
    from concourse.bass_utils import run_bass_kernel_spmd

    if "nc" not in _cache:
        _cache["nc"] = _build_module()
    nc = _cache["nc"]

    shared = _host_prep(inputs)
    xfull = np.asarray(inputs["input"], np.float32)  # [B, S, D]
    in_maps = []
    for c in range(NCORES):
        m = dict(shared)
        m["xin"] = np.ascontiguousarray(
            xfull[c * BL:(c + 1) * BL].reshape(TOK, D))
        in_maps.append(m)

    res = run_bass_kernel_spmd(nc, in_maps, core_ids=list(range(NCORES)))
    out = np.empty((B, S, D), np.float32)
    for c in range(NCORES):
        out[c * BL:(c + 1) * BL] = res.results[c]["xout"].reshape(BL, S, D)
    return out
